# revision 26
# baseline (speedup 1.0000x reference)
"""KAN Fourier-linear kernel for 8 Trainium2 NeuronCores.

y[n,o] = sum_{i,g} C0[o,i,g]*cos(g*x[n,i]) + C1[o,i,g]*sin(g*x[n,i]) + bias[o]

Strategy (data-parallel over n, 4096 rows/core):
  - Base features cos/sin(g*x) for g in {1,2,4} via range-reduce + ACT Sin:
      v   = int32(x*a_g + C_g)            # gpsimd tensor_scalar
      r_g = x - v*(2pi/g)                 # DVE scalar_tensor_tensor (fp32)
      f   = Sin(scale=g, bias=b)(r_g)     # ACT spline, arg in [-5pi/4, 3pi/4]
  - Remaining 26 features as single DVE bf16 multiplies using
      f * cos(b*x) = (f_{a+b} + f_{a-b}) / 2     (plain-cos multipliers)
    with cos(8x) built as 1 - 2*sin(4x)^2; all correction terms and scale
    factors folded into the weights on the host (solve M^T W_comp = W_true
    for the 32x32 expansion matrix M).
  - y.T tile = W.T @ F via PE, K=4096 accumulated in PSUM (bf16 inputs).
  - 4 superpasses of 1024 cols, per-(oh,chi) single-bank PSUM tiles with
    pool rotation, chunked DMAs on the sync ring, PE warm-up matmuls, ACT
    table preload, oh-major matmul order on the last superpass.
"""
import math
import numpy as np
from contextlib import ExitStack

import concourse.bass as bass
import concourse.mybir as mybir
import concourse.tile as tile
from concourse import bacc
from concourse.bass_utils import run_bass_kernel_spmd

import ml_dtypes

N_CORES = 8
N_TOTAL = 32768
N_SHARD = N_TOTAL // N_CORES        # 4096 rows per core
INDIM = 128
OUTDIM = 256
GRID = 16
SP = 4                              # n-superpasses per core
S = N_SHARD // SP                   # 1024 columns per superpass
CH = 512                            # matmul moving chunk
TWO_PI = 2.0 * math.pi

FP32 = mybir.dt.float32
BF16 = mybir.dt.bfloat16
I32 = mybir.dt.int32

BASE = [1, 2, 4]
# tile spec: (name, kind, *args); kinds: act-c, act-s, chalf, mul, sq.
# Multipliers are plain cos tiles (amplitude-preserving): f*cos(b) =
# (f_{a+b} + f_{a-b})/2; the 1/2 and all corrections fold into the weights.
TILE_SPEC = [
    ("c1", "act-c", 1), ("s1", "act-s", 1),
    ("c2", "act-c", 2), ("s2", "act-s", 2),
    ("c4", "act-c", 4), ("s4", "act-s", 4),
    ("Lc3", "mul", "c2", "c1"), ("Ls3", "mul", "s2", "c1"),
    ("c8x", "sq", "s4", 8), ("s8h", "mul", "s4", "c4"),
    ("ch8", "chalf", "c8x", 8),
    ("Lc5", "mul", "c1", "c4"), ("Ls5", "mul", "s1", "c4"),
    ("Lc6", "mul", "c2", "c4"), ("Ls6", "mul", "s2", "c4"),
    ("Lc7", "mul", "Lc3", "c4"), ("Ls7", "mul", "Ls3", "c4"),
    ("Lc9", "mul", "c1", "ch8"), ("Ls9", "mul", "s1", "ch8"),
    ("Lc10", "mul", "c2", "ch8"), ("Ls10", "mul", "s2", "ch8"),
    ("Lc11", "mul", "Lc3", "ch8"), ("Ls11", "mul", "Ls3", "ch8"),
    ("Lc12", "mul", "c4", "ch8"), ("Ls12", "mul", "s4", "ch8"),
    ("Lc16", "mul", "c8x", "ch8"), ("Ls16", "mul", "s8h", "ch8"),
    ("Lc13", "mul", "Lc5", "ch8"), ("Ls13", "mul", "Ls5", "ch8"),
    ("Lc14", "mul", "Lc6", "ch8"), ("Ls14", "mul", "Ls6", "ch8"),
    ("Lc15", "mul", "Lc7", "ch8"), ("Ls15", "mul", "Ls7", "ch8"),
]
CP_B = {"c1": 1, "c2": 2, "c4": 4, "ch8": 8}
# PSUM accumulation (= weight) order, chosen so consumption tracks production.
KT_ORDER = ["c1", "s1", "c2", "s2", "Lc3", "Ls3", "c4", "s4",
            "Lc5", "Ls5", "Lc6", "Ls6", "Lc7", "Ls7", "s8h", "c8x",
            "Lc9", "Ls9", "Lc10", "Ls10", "Lc11", "Ls11",
            "Lc12", "Ls12", "Lc16", "Ls16",
            "Lc13", "Ls13", "Lc14", "Ls14", "Lc15", "Ls15"]
# leaf tiles that feed further products need double buffering
SRC_TILES = {"Lc3", "Ls3", "c8x", "s8h", "ch8",
             "Lc5", "Ls5", "Lc6", "Ls6", "Lc7", "Ls7"}


def _g_consts(g: int):
    a = np.float32(g / TWO_PI)
    phat = np.float32(TWO_PI / g)
    m = 2.0 ** math.ceil(math.log2(0.960 * g + 0.14))
    c = np.float32(m + 0.125)
    b_s = np.float32(m * g * float(phat))      # == 2pi*m up to fp32, matched to phat
    b_c = np.float32(float(b_s) + math.pi / 2.0)
    return a, phat, c, b_s, b_c


def _expansions():
    """Map each computed tile to its exact expansion over true features."""
    def expand_mult(expA, b):
        out = {}

        def add(k, v):
            out[k] = out.get(k, 0.0) + v

        for k, coef in expA.items():
            coef = coef * 0.5
            if k == "const":
                add(("c", b), 2.0 * coef)
                continue
            t, g = k
            hi, lo = g + b, g - b
            if t == "c":
                add(("c", hi), coef)
                if lo == 0:
                    add("const", coef)
                else:
                    add(("c", abs(lo)), coef)
            else:
                add(("s", hi), coef)
                if lo != 0:
                    add(("s", abs(lo)), coef if lo > 0 else -coef)
        return {k: v for k, v in out.items() if v != 0.0}

    exps = {}
    for spec in TILE_SPEC:
        name, kind = spec[0], spec[1]
        if kind == "act-c":
            exps[name] = {("c", spec[2]): 1.0}
        elif kind == "act-s":
            exps[name] = {("s", spec[2]): 1.0}
        elif kind == "chalf":
            exps[name] = {("c", spec[3]): 1.0}       # 1 - 2*sin(b/2 x)^2
        elif kind == "sq":
            exps[name] = {"const": 0.5, ("c", spec[3]): -0.5}
        elif kind == "mul":
            exps[name] = expand_mult(exps[spec[2]], CP_B[spec[3]])
    return exps


def _expansion_matrix():
    exps = _expansions()
    M = np.zeros((32, 32))
    m0 = np.zeros(32)
    for kt, key in enumerate(KT_ORDER):
        for k, coef in exps[key].items():
            if k == "const":
                m0[kt] = coef
            else:
                t, g = k
                M[kt, 2 * (g - 1) + (0 if t == "c" else 1)] = coef
    return M, m0


_CACHED = {}


def _build():
    if "nc" in _CACHED:
        return _CACHED["nc"]
    nc = bacc.Bacc("TRN2", target_bir_lowering=False, debug=False,
                   num_devices=N_CORES)
    xt_d = nc.dram_tensor("xt", [INDIM, N_SHARD], FP32, kind="ExternalInput").ap()
    w_d = nc.dram_tensor("w", [INDIM, 32 * OUTDIM], BF16, kind="ExternalInput").ap()
    bt_d = nc.dram_tensor("bt", [INDIM, 2 * len(BASE)], FP32, kind="ExternalInput").ap()
    bias_d = nc.dram_tensor("bias", [INDIM, 2], FP32, kind="ExternalInput").ap()
    yt_d = nc.dram_tensor("yt", [OUTDIM, N_SHARD], FP32, kind="ExternalOutput").ap()

    with tile.TileContext(nc) as tc, ExitStack() as ctx:
        cpool = ctx.enter_context(tc.tile_pool(name="const", bufs=1))
        vpool = ctx.enter_context(tc.tile_pool(name="v", bufs=3))
        rpool = ctx.enter_context(tc.tile_pool(name="r", bufs=3))
        bpool = ctx.enter_context(tc.tile_pool(name="base", bufs=2))
        lpool = ctx.enter_context(tc.tile_pool(name="leaf", bufs=1))
        ypool = ctx.enter_context(tc.tile_pool(name="y", bufs=2))
        ppool = ctx.enter_context(tc.tile_pool(name="psum", bufs=2, space="PSUM"))

        xt = cpool.tile([INDIM, N_SHARD], FP32)
        wt = cpool.tile([INDIM, 32 * OUTDIM], BF16)
        bt = cpool.tile([INDIM, 2 * len(BASE)], FP32)
        bias = cpool.tile([INDIM, 2], FP32)

        scratch = cpool.tile([128, CH], BF16)
        garb = cpool.tile([128, 8], BF16)
        nc.vector.memset(scratch[:], 0)
        # Preloads the Sin table set (walrus inserts ACT_TABLE_LOAD before
        # this) so the first real activation doesn't pay the ~1.3us load.
        nc.scalar.activation(garb[:], scratch[:, 0:8],
                             mybir.ActivationFunctionType.Sin, bias=0.0)

        nc.sync.dma_start(xt[:, 0:S // 2], xt_d[:, 0:S // 2])
        nc.sync.dma_start(xt[:, S // 2:S], xt_d[:, S // 2:S])
        nc.sync.dma_start(bt[:], bt_d[:])
        nc.sync.dma_start(bias[:], bias_d[:])
        nc.sync.dma_start(wt[:, 0:512], w_d[:, 0:512])
        nc.sync.dma_start(wt[:, 512:2048], w_d[:, 512:2048])
        for j in range(1, 4):
            nc.sync.dma_start(xt[:, j * S:(j + 1) * S],
                              xt_d[:, j * S:(j + 1) * S])
            nc.sync.dma_start(wt[:, j * 2048:(j + 1) * 2048],
                              w_d[:, j * 2048:(j + 1) * 2048])

        # PE warm-up: HAM un-throttles after ~3.4us of sustained activity.
        # Burn dummy matmuls on the zeroed scratch tile while input DMAs land
        # so the real matmul stream starts at 2.4 GHz.
        pwarm = ppool.tile([128, CH], FP32, tag="p00", name="pwarm")
        for _ in range(12):
            nc.tensor.matmul(pwarm[:], scratch[:, 0:128], scratch[:],
                             start=True, stop=True)

        psums_of = {}

        def emit_feats(sp):
            xs = xt[:, sp * S:(sp + 1) * S]
            g1_halves = ([(0, S // 2), (S // 2, S)] if sp == 0 else [(0, S)])
            cst = {g: _g_consts(g) for g in BASE}
            tiles = {}
            v_t, r_t = {}, {}

            def emit_v(g):
                v = vpool.tile([INDIM, S], I32, tag="v", name=f"v{g}")
                for h0, h1 in (g1_halves if g == 1 else [(0, S)]):
                    nc.gpsimd.tensor_scalar(
                        v[:, h0:h1], xs[:, h0:h1],
                        float(cst[g][0]), float(cst[g][2]),
                        mybir.AluOpType.mult, mybir.AluOpType.add)
                v_t[g] = v

            def emit_r(g):
                r = rpool.tile([INDIM, S], FP32, tag="r", name=f"r{g}")
                for h0, h1 in (g1_halves if g == 1 else [(0, S)]):
                    nc.vector.scalar_tensor_tensor(
                        r[:, h0:h1], v_t[g][:, h0:h1],
                        float(-cst[g][1]), xs[:, h0:h1],
                        mybir.AluOpType.mult, mybir.AluOpType.add)
                r_t[g] = r

            def emit_act(t, g):
                gi = BASE.index(g)
                col = 2 * gi + (0 if t == "c" else 1)
                f = bpool.tile([INDIM, S], BF16, tag=f"{t}{g}", name=f"{t}{g}")
                for h0, h1 in (g1_halves if g == 1 else [(0, S)]):
                    nc.scalar.activation(f[:, h0:h1], r_t[g][:, h0:h1],
                                         mybir.ActivationFunctionType.Sin,
                                         bias=bt[:, col:col + 1],
                                         scale=float(g))
                tiles[f"{t}{g}"] = f

            def emit_derived(name):
                spec = next(s for s in TILE_SPEC if s[0] == name)
                kind = spec[1]
                pool = bpool if name in SRC_TILES else lpool
                f = pool.tile([INDIM, S], BF16, tag=name, name=name)
                if kind == "mul":
                    nc.vector.tensor_mul(f[:], tiles[spec[2]][:],
                                         tiles[spec[3]][:])
                elif kind == "sq":
                    nc.vector.tensor_mul(f[:], tiles[spec[2]][:],
                                         tiles[spec[2]][:])
                elif kind == "chalf":
                    nc.vector.tensor_scalar(f[:], tiles[spec[2]][:],
                                            -2.0, 1.0,
                                            mybir.AluOpType.mult,
                                            mybir.AluOpType.add)
                tiles[name] = f

            emit_v(1)
            emit_v(2)
            emit_v(4)
            emit_r(1)
            emit_r(2)
            emit_r(4)
            for t, g in (("c", 1), ("s", 1), ("c", 2), ("s", 2),
                         ("c", 4), ("s", 4)):
                emit_act(t, g)
            for name in ("Lc3", "Ls3", "Lc5", "Ls5", "Lc6", "Ls6",
                         "Lc7", "Ls7", "c8x", "s8h", "ch8",
                         "Lc9", "Ls9", "Lc10", "Ls10", "Lc11", "Ls11",
                         "Lc12", "Ls12", "Lc16", "Ls16",
                         "Lc13", "Ls13", "Lc14", "Ls14", "Lc15", "Ls15"):
                emit_derived(name)
            return tiles

        def emit_mms(sp, tiles, oh_major=False):
            psums = {}
            for oh in range(2):
                for chi in range(S // CH):
                    psums[(oh, chi)] = ppool.tile(
                        [128, CH], FP32, tag=f"p{oh}{chi}",
                        name=f"psum{oh}{chi}")
            psums_of[sp] = psums

            def mm(kt, oh, chi):
                lhsT = wt[:, kt * OUTDIM + oh * 128:
                          kt * OUTDIM + oh * 128 + 128]
                nc.tensor.matmul(
                    psums[(oh, chi)][:],
                    lhsT, tiles[KT_ORDER[kt]][:, chi * CH:(chi + 1) * CH],
                    start=(kt == 0), stop=(kt == 31),
                )

            if oh_major:
                # Last superpass: finish the oh0 groups 64 MMs early so
                # their PSUM drain + output DMA overlap the oh1 groups.
                for oh in range(2):
                    for kt in range(len(KT_ORDER)):
                        for chi in range(S // CH):
                            mm(kt, oh, chi)
            else:
                for kt in range(len(KT_ORDER)):
                    for oh in range(2):
                        for chi in range(S // CH):
                            mm(kt, oh, chi)

        def emit_out(sp):
            for oh in range(2):
                for chi in range(S // CH):
                    y = ypool.tile([128, CH], FP32, tag=f"y{oh}{chi}",
                                   name=f"y{oh}{chi}")
                    nc.scalar.activation(
                        y[:], psums_of[sp][(oh, chi)][:],
                        mybir.ActivationFunctionType.Identity,
                        bias=bias[:, oh:oh + 1])
                    # oh1 (the last-finishing groups) issue from the sync
                    # ring so the DMA issue doesn't sit on the ACT queue
                    # between the two final Identity drains.
                    ring = nc.scalar if oh == 0 and chi == 0 else nc.sync
                    ring.dma_start(
                        yt_d[oh * 128:(oh + 1) * 128,
                             sp * S + chi * CH:sp * S + (chi + 1) * CH], y[:])

        tiles = emit_feats(0)
        emit_mms(0, tiles)
        for sp in range(1, SP):
            tiles = emit_feats(sp)
            emit_out(sp - 1)
            emit_mms(sp, tiles, oh_major=(sp == SP - 1))
        emit_out(SP - 1)

    nc.compile()
    _CACHED["nc"] = nc
    return nc


def _prep_inputs(x: np.ndarray, fouriercoeffs: np.ndarray, bias: np.ndarray):
    xt = np.ascontiguousarray(x.astype(np.float32, copy=False).T)  # (128, 32768)

    M, m0 = _expansion_matrix()
    fc = fouriercoeffs.astype(np.float64, copy=False)
    W_true = np.zeros((OUTDIM, INDIM, 32))
    for g in range(1, GRID + 1):
        W_true[:, :, 2 * (g - 1)] = fc[0, :, :, g - 1]
        W_true[:, :, 2 * (g - 1) + 1] = fc[1, :, :, g - 1]
    W_comp = np.linalg.solve(M.T, W_true.reshape(-1, 32).T).T.reshape(
        OUTDIM, INDIM, 32)
    # w_sb[i, kt*256 + o] = W_comp[o, i, kt]
    w_sb = np.ascontiguousarray(
        W_comp.transpose(1, 2, 0).reshape(INDIM, 32 * OUTDIM)
    ).astype(ml_dtypes.bfloat16)

    bias_new = bias.reshape(-1).astype(np.float64) - np.einsum(
        "oik,k->o", W_comp, m0)

    bvals = np.empty(2 * len(BASE), np.float32)
    for gi, g in enumerate(BASE):
        _, _, _, b_s, b_c = _g_consts(g)
        bvals[2 * gi] = b_c
        bvals[2 * gi + 1] = b_s
    bt = np.tile(bvals[None, :], (INDIM, 1)).astype(np.float32)
    bias_sb = np.ascontiguousarray(
        bias_new.reshape(2, 128).T.astype(np.float32))      # (128, 2)
    return xt, w_sb, bt, bias_sb


def kernel(x: np.ndarray, fouriercoeffs: np.ndarray, bias: np.ndarray,
           _trace: bool = False):
    x = np.asarray(x)
    fouriercoeffs = np.asarray(fouriercoeffs)
    bias = np.asarray(bias)
    orig_shape = x.shape
    x2 = x.reshape(-1, INDIM)
    assert x2.shape == (N_TOTAL, INDIM), x2.shape

    nc = _build()
    xt, w_sb, bt, bias_sb = _prep_inputs(x2, fouriercoeffs, bias)
    in_maps = []
    for c in range(N_CORES):
        in_maps.append({
            "xt": np.ascontiguousarray(xt[:, c * N_SHARD:(c + 1) * N_SHARD]),
            "w": w_sb,
            "bt": bt,
            "bias": bias_sb,
        })
    res = run_bass_kernel_spmd(nc, in_maps, list(range(N_CORES)),
                               trace=_trace)
    yt = np.concatenate([res.results[c]["yt"] for c in range(N_CORES)], axis=1)
    y = np.ascontiguousarray(yt.T).astype(np.float32)
    if _trace:
        kernel._last_result = res
    return y.reshape(*orig_shape[:-1], OUTDIM)
